# revision 65
# baseline (speedup 1.0000x reference)
"""Trainium2 Bass kernel: 16-head causal self-attention block (QKV proj ->
causal MHA -> output proj), tensor-parallel over heads across 8 NeuronCores.

Contract: kernel(**inputs) takes FULL unsharded inputs
  x      [2, 2048, 1024] f32
  w_qkv  [1024, 3072] f32, b_qkv [3072] f32
  w_proj [1024, 1024] f32, b_proj [1024] f32
and returns the FULL output [2, 2048, 1024] f32.

Sharding: head-parallel. Core c owns global heads (2c, 2c+1):
  - column-parallel QKV (each core takes its 128 q/k/v feature columns)
  - full causal attention for its 2 heads (both batches)
  - row-parallel output projection -> partial [4096, 1024] sums
  - host reduces the 8 partials and adds b_proj.

v2 dataflow (vs v1): software-pipelined emission interleaves QKV of
super-tile s with attention of super s-1 so the PE never starves while
ScalarE runs softmax exp (ScalarE does ONLY exp + a few psum drains; no
Ln -> no act-table thrash). q/k/P/v all fp16 on the PE. v is computed in
natural [token, feat] orientation directly (xT stationary), killing the
PE transposes; v bias folds in at the PSUM drain via a precomputed
broadcast tile. x is pre-transposed on the host so all x loads are plain
contiguous DMAs.

v3 (vs v2, 217.7us -> ~168.5us):
- 1/Z via single-op DVE reciprocal_approx_fast on an SBUF bounce of the
  replicated Z ones-rows of the AV PSUM accumulator (drops the
  Z-broadcast matmul and 30us of exact DVE reciprocal from the
  normalize critical path; the custom op cannot read PSUM directly).
- q/k bias drains moved to ScalarE activation(Identity, bias=AP): DVE
  decongests and the next attention tile's S can start sooner.
- per-tt v PSUM tiles double-buffer the v chains; causal mask-muls on
  the Pool engine (SBUF-only op); all proj drains on DVE.
- output partials drain as fp16 (halves the out-DMA that otherwise
  piles up at the tail).
- tail: supers 4,5 proj deferred into the exp-paced tail via poolY
  chunk drains; super 6's proj is a reserve pulled inline by the last
  q-tile, which pipelines its normalize+proj in 256-col halves with
  Z copies on the idle ScalarE and DMA spread over two queues. This
  keeps the PE dense enough that the HAM clock holds k=8 until the
  final matmul retires.
"""

import numpy as np
from contextlib import ExitStack

import concourse.bass as bass
import concourse.tile as tile
from concourse import bacc, mybir
from concourse.bass_utils import run_bass_kernel_spmd
from concourse.masks import make_upper_triangular

F32 = mybir.dt.float32
F32R = mybir.dt.float32r
F16 = mybir.dt.float16
AF = mybir.ActivationFunctionType

N_CORES = 8
B, T, E, H, D = 2, 2048, 1024, 16, 64
TOK = B * T          # 4096 tokens
P = 128              # partitions
SUPER = 512          # tokens per QKV super-tile
NS = TOK // SUPER    # 8 super-tiles
KCH = E // P         # 8 contraction chunks
QTL = 512            # attention q-tile width
NQT = T // QTL       # 4 q-tiles per batch
KBL = 128            # attention k-block height
VAW = 256            # v_aug cols per token tile: 2 heads x (64 v + 64 ones);
                     # the 64 replicated ones-columns make the AV matmul emit
                     # the softmax denominator Z broadcast on partitions 64-127


def r(ap):
    return ap.bitcast(F32R)


def _emit(nc, tc, ctx):
    # weights arrive host-relayouted: [P, E] chunk-major, one contiguous DMA
    xT_h = nc.declare_dram_parameter("xT", [E, TOK], F16, isOutput=False)
    wq_h = nc.declare_dram_parameter("wq", [P, E], F16, isOutput=False)
    wk_h = nc.declare_dram_parameter("wk", [P, E], F16, isOutput=False)
    wv_h = nc.declare_dram_parameter("wv", [P, E], F16, isOutput=False)
    bqk_h = nc.declare_dram_parameter("bqk", [P, 2], F32, isOutput=False)
    bv_h = nc.declare_dram_parameter("bv", [1, P], F16, isOutput=False)
    wp_h = nc.declare_dram_parameter("wp", [P, E], F16, isOutput=False)
    out_h = nc.declare_dram_parameter("out", [TOK, E], F16, isOutput=True)

    outr = out_h[:].rearrange("(n p) e -> n p e", p=P)  # [32, 128, 1024]

    # ---------------- persistent tiles ----------------
    const = ctx.enter_context(tc.tile_pool(name="const", bufs=1))
    mask_tri = const.tile([P, P], F16)  # mask[p, f] = 1.0 iff p <= f
    make_upper_triangular(nc, mask_tri[:], val=1.0, diag=True)
    ones1h = const.tile([1, P], F16)
    nc.vector.memset(ones1h[:], 1.0)


    # weight/bias loads go on the Activation queue (idle at startup) so the
    # x loads on the sync queue aren't delayed behind them
    bqk_sb = const.tile([P, 2], F32)
    bvr = const.tile([1, P], F16)
    nc.scalar.dma_start(bqk_sb[:], bqk_h[:])
    nc.scalar.dma_start(bvr[:], bv_h[:])
    bq_sb = bqk_sb[:, 0:1]
    bk_sb = bqk_sb[:, 1:2]
    wq_sb = const.tile([P, E], F16)
    wk_sb = const.tile([P, E], F16)
    wv_sb = const.tile([P, E], F16)
    wp_sb = const.tile([P, E], F16)
    # one weight per queue: a single queue moves these serially at ~100GB/s
    # and the first q-chain would wait ~10us for wq otherwise
    for wsb, wh in ((wq_sb, wq_h), (wk_sb, wk_h), (wv_sb, wv_h),
                    (wp_sb, wp_h)):
        nc.scalar.dma_start(wsb[:], wh[:])
    bvb = const.tile([P, P], F32)  # v bias broadcast to all 128 partitions

    persist = ctx.enter_context(tc.tile_pool(name="persist", bufs=1))

    with ExitStack() as ph:
        xpool = ph.enter_context(tc.tile_pool(name="xp", bufs=4))
        ptpool = ph.enter_context(tc.tile_pool(name="pTp", bufs=12))
        zbpool = ph.enter_context(tc.tile_pool(name="zbp", bufs=2))
        opool = ph.enter_context(tc.tile_pool(name="op", bufs=6))
        poolQ = ph.enter_context(tc.tile_pool(name="poolQ", bufs=2, space="PSUM"))
        poolS = ph.enter_context(tc.tile_pool(name="poolS", bufs=2, space="PSUM"))
        poolY = ph.enter_context(tc.tile_pool(name="poolY", bufs=2, space="PSUM"))

        # v bias broadcast: bvb[p, j] = bv[j] via rank-1 matmul of ones x bv
        pb = poolQ.tile([P, P], F32, tag="q", name="pb")
        # HAM warmup: the PE would otherwise idle ~6us waiting for the weight
        # DMAs and start the real QKV chains at half clock (K=4/8). A burst
        # of dependency-free matmuls (ones x ones, overwritten below) keeps
        # the PE activity window busy so the clock-gate opens first.
        for _ in range(48):
            nc.tensor.matmul(pb[0:32, :], lhsT=ones1h[0:1, 0:32],
                             rhs=ones1h[:], start=True, stop=True)
        nc.tensor.matmul(pb[:], lhsT=ones1h[:], rhs=bvr[:], start=True,
                         stop=True)
        nc.vector.tensor_copy(bvb[:], pb[:])

        yts = [persist.tile([P, QTL], F16, tag=f"yt{n}", name="yt")
               for n in range(NS)]
        projq = []
        qTs, kTs, vas = [], [], []
        for s in range(NS):
            qTs.append(persist.tile([P, SUPER], F16, tag=f"qT{s}", name="qTt"))
            kTs.append(persist.tile([P, SUPER], F16, tag=f"kT{s}", name="kTt"))
            vat = persist.tile([P, 4 * VAW], F16, tag=f"va{s}", name="vat")
            vas.append(vat)
            # whole tile starts as ones; the per-super v drains overwrite the
            # v columns, leaving the replicated ones-columns (64-127 of each
            # head group) that make the AV matmul emit Z on partitions 64-127
            nc.gpsimd.memset(vat[:], 1.0)

        xtiles = {}
        xT_cm = xT_h[:].rearrange("(c p) tok -> p c tok", p=P)  # [128,8,4096]

        def issue_x(s):
            # one strided DMA per super-tile; chunk ch lands contiguous at
            # cols [ch*512, (ch+1)*512). The first super is split in half so
            # its q-chain can start on chunks 0-3 while 4-7 are in flight.
            xt = xpool.tile([P, KCH * SUPER], F16, tag="xT", name="xTt")
            nc.sync.dma_start(
                xt[:].rearrange("p (c t) -> p c t", c=KCH),
                xT_cm[:, :, s * SUPER:(s + 1) * SUPER])
            xtiles[s] = [xt[:, ch * SUPER:(ch + 1) * SUPER]
                         for ch in range(KCH)]

        issue_x(0)
        issue_x(1)
        issue_x(2)

        def qkv_units(s):
            xt = xtiles[s]
            if s + 3 < NS:
                issue_x(s + 3)
            yield
            # q chain fully before k chain: pfq stops ~2us earlier, so its
            # ScalarE drain overlaps the k chain and the poolQ slot is
            # already free when the v chains (and the next super's q) need
            # it -- this was the recurring super-boundary PE stall
            pfq = poolQ.tile([P, SUPER], F32, tag="q", name="pfq")
            for ch in range(KCH):
                nc.tensor.matmul(
                    pfq[:], lhsT=wq_sb[:, ch * P:(ch + 1) * P],
                    rhs=xt[ch][:], start=(ch == 0), stop=(ch == KCH - 1))
                if ch % 2 == 1:
                    yield
            # drain on ScalarE (bias folds into the activation) so the
            # attention S-start never queues behind DVE
            nc.scalar.activation(qTs[s][:], pfq[:], AF.Identity, bias=bq_sb)
            pfk = poolQ.tile([P, SUPER], F32, tag="q", name="pfk")
            for ch in range(KCH):
                nc.tensor.matmul(
                    pfk[:], lhsT=wk_sb[:, ch * P:(ch + 1) * P],
                    rhs=xt[ch][:], start=(ch == 0), stop=(ch == KCH - 1))
                if ch % 2 == 1:
                    yield
            # k drains on DVE (front-loading already cleared its queue
            # here): keeps ScalarE free to stream exps at the boundary
            nc.vector.tensor_scalar_add(kTs[s][:], pfk[:], bk_sb)
            yield
            bvb2 = bvb[:].rearrange("p (h d) -> p h d", h=2)
            for tt in range(4):
                vps = poolQ.tile([P, P], F32, tag="q", name="vps")
                for ch in range(KCH):
                    nc.tensor.matmul(
                        vps[:],
                        lhsT=xt[ch][:, tt * P:(tt + 1) * P],
                        rhs=wv_sb[:, ch * P:(ch + 1) * P],
                        start=(ch == 0), stop=(ch == KCH - 1))
                dst = vas[s][:, tt * VAW:(tt + 1) * VAW].rearrange(
                    "p (h x) -> p h x", x=2 * D)[:, :, 0:D]
                src = vps[:].rearrange("p (h d) -> p h d", h=2)
                nc.vector.tensor_add(dst, src, bvb2)
                yield

        projq = []

        def attn_units(b, qi):
            nkb = 4 * qi + 4   # k blocks of 128 covering [0, (qi+1)*512)
            sq = 4 * b + qi    # super-tile holding this q range
            # the last q-tile takes its accumulators from poolQ (idle once
            # QKV is done) so its blocks overlap the previous q-tile's tail
            ypool, ytag = (poolQ, "q") if sq == NS - 1 else (poolY, "y")
            pys = [ypool.tile([P, QTL], F32, tag=ytag, name=f"py{h}")
                   for h in range(2)]

            def emit_S(kb):
                c0 = max(0, kb * KBL - qi * QTL)
                sk, kc = 4 * b + kb // 4, (kb % 4) * KBL
                ps = poolS.tile([P, 2 * QTL], F32, tag="s", name="ps")
                for h in range(2):
                    nc.tensor.matmul(
                        ps[:, h * QTL + c0:(h + 1) * QTL],
                        lhsT=kTs[sk][64 * h:64 * h + 64, kc:kc + KBL],
                        rhs=qTs[sq][64 * h:64 * h + 64, c0:QTL],
                        start=True, stop=True)
                return ps, c0

            cur = emit_S(0)
            for kb in range(nkb):
                ps, c0 = cur
                if kb + 1 < nkb:
                    cur = emit_S(kb + 1)
                pt = ptpool.tile([P, 2 * QTL], F16, tag="pT", name="pt")
                if c0 == 0:
                    nc.scalar.activation(pt[:], ps[:], AF.Exp, scale=0.125)
                else:
                    src = ps[:].rearrange("p (h q) -> p h q", h=2)[:, :, c0:]
                    dst = pt[:].rearrange("p (h q) -> p h q", h=2)[:, :, c0:]
                    nc.scalar.activation(dst, src, AF.Exp, scale=0.125)
                if kb * KBL >= qi * QTL:  # diagonal block: causal triangle
                    sl = pt[:].rearrange("p (h q) -> p h q",
                                         h=2)[:, :, c0:c0 + P]
                    m3 = mask_tri[:].rearrange(
                        "p (u f) -> p u f", u=1).broadcast_to([P, 2, P])
                    # SBUF-only op: run it on the (otherwise idle) Pool
                    # engine so DVE keeps its bandwidth for PSUM drains
                    nc.gpsimd.tensor_mul(sl, sl, m3)
                vo = (kb % 4) * VAW
                sk = 4 * b + kb // 4
                for h in range(2):
                    nc.tensor.matmul(
                        pys[h][:, c0:QTL],
                        lhsT=vas[sk][:, vo + 2 * D * h:vo + 2 * D * h + 2 * D],
                        rhs=pt[:, h * QTL + c0:(h + 1) * QTL],
                        start=(kb == 0), stop=(kb == nkb - 1))
                # keep-warm: a dependency-free weight load keeps the PE's
                # HAM activity window busy through exp-wait micro-idles
                nc.tensor.ldweights(weights=mask_tri[0:1, 0:32])
                yield
            # normalize: y * (1/Z). pys[h] partitions 64-127 already hold Z
            # broadcast (the replicated ones-columns), so a fast approximate
            # reciprocal reads them straight out of PSUM -- no Z-broadcast
            # matmul, no exact (slow) reciprocal on the critical path. The
            # custom-DVE recip needs raw fp32 bits and PSUM reads break its
            # BITWISE_NOT seed, so bounce Z through SBUF first.
            yt = yts[sq]
            if sq == NS - 1:
                # last q-tile: nothing overlaps its tail, so pipeline
                # normalize+proj in 256-col halves to keep the PE fed and
                # the final out-DMA issued as early as possible. PE filler
                # during the Z-drain latency comes from the reserve (the
                # last deferred super's proj units); the Z copies go on
                # ScalarE, which is idle once the final exp retires.
                def pull(n):
                    while n > 0 and reserve:
                        try:
                            next(reserve[0])
                            n -= 1
                        except StopIteration:
                            reserve.pop(0)
                pull(3)
                for half in range(2):
                    hs = slice(half * 2 * P, (half + 1) * 2 * P)
                    zs = zbpool.tile([P, 2 * P], F32, tag="zb", name="zs")
                    # one Z copy per engine so they run in parallel
                    nc.vector.tensor_copy(zs[0:D, :], pys[0][D:2 * D, hs])
                    nc.scalar.activation(zs[D:2 * D, :], pys[1][D:2 * D, hs],
                                         AF.Copy)
                    zinv = zbpool.tile([P, 2 * P], F32, tag="zb", name="zinv")
                    nc.vector.reciprocal_approx_fast(zinv[:], zs[:])
                    nc.vector.tensor_mul(yt[0:D, hs], pys[0][0:D, hs],
                                         zinv[0:D, :])
                    nc.vector.tensor_mul(yt[D:2 * D, hs], pys[1][0:D, hs],
                                         zinv[D:2 * D, :])
                    pull(1)
                    yield
                    for tt4 in (2 * half, 2 * half + 1):
                        cs = slice(tt4 * P, (tt4 + 1) * P)
                        pos = poolS.tile([P, E], F32, tag="s", name="po")
                        for oc in range(2):
                            nc.tensor.matmul(
                                pos[:, oc * 512:(oc + 1) * 512],
                                lhsT=yt[:, cs],
                                rhs=wp_sb[:, oc * 512:(oc + 1) * 512],
                                start=True, stop=True)
                        ti = (b * T + qi * QTL) // P + tt4
                        ot = opool.tile([P, E], F16, tag="ot", name="ot")
                        # drain halves on DVE + ScalarE (idle after the
                        # final exp) in parallel, and DMA each half as soon
                        # as its drain lands, on separate queues -- the
                        # kernel cannot retire until the last byte is out
                        nc.vector.tensor_copy(ot[:, 0:512], pos[:, 0:512])
                        nc.scalar.activation(ot[:, 512:1024],
                                             pos[:, 512:1024], AF.Copy)
                        nc.gpsimd.dma_start(outr[ti][:, 0:512],
                                            ot[:, 0:512])
                        nc.sync.dma_start(outr[ti][:, 512:1024],
                                          ot[:, 512:1024])
                        yield
                return
            zs = zbpool.tile([P, QTL], F32, tag="zb", name="zs")
            nc.vector.tensor_copy(zs[0:D, :], pys[0][D:2 * D, :])
            nc.vector.tensor_copy(zs[D:2 * D, :], pys[1][D:2 * D, :])
            zinv = zbpool.tile([P, QTL], F32, tag="zb", name="zinv")
            nc.vector.reciprocal_approx_fast(zinv[:], zs[:])
            yield
            nc.vector.tensor_mul(yt[0:D, :], pys[0][0:D, :], zinv[0:D, :])
            nc.vector.tensor_mul(yt[D:2 * D, :], pys[1][0:D, :],
                                 zinv[D:2 * D, :])
            yield
            if sq in (4, 5):
                # deferred: dense PE work reserved for the exp-paced tail so
                # the HAM clock stays warm through the last q-tiles
                projq.append(proj_units(b, qi))
            elif sq == 6:
                # super 6's proj is the reserve the last q-tile pulls from
                reserve.append(proj_units(b, qi, tail=True))
            else:
                yield from proj_units(b, qi)

        def proj_units(b, qi, tail=False):
            yt = yts[4 * b + qi]
            # mid-tail-deferred tiles take their PSUM from poolY (idle in
            # the tail: the last q-tile's accumulators live in poolQ), so
            # the tail's S-block double-buffer in poolS never blocks on a
            # proj drain. The reserve tiles pulled during the last q-tile's
            # normalize use poolS (S-blocks are done by then) and drain in
            # halves on DVE+ScalarE so they never queue in front of the
            # normalize chain on DVE.
            deferred = 4 * b + qi in (4, 5)
            for tt4 in range(4):
                cs = slice(tt4 * P, (tt4 + 1) * P)
                ti = (b * T + qi * QTL) // P + tt4
                ot = opool.tile([P, E], F16, tag="ot", name="ot")
                if tail:
                    pos = poolS.tile([P, E], F32, tag="s", name="po")
                    for oc in range(2):
                        nc.tensor.matmul(
                            pos[:, oc * 512:(oc + 1) * 512],
                            lhsT=yt[:, cs],
                            rhs=wp_sb[:, oc * 512:(oc + 1) * 512],
                            start=True, stop=True)
                    nc.vector.tensor_copy(ot[:, 0:512], pos[:, 0:512])
                    nc.scalar.activation(ot[:, 512:1024], pos[:, 512:1024],
                                         AF.Copy)
                    dq = nc.sync if tt4 % 2 else nc.gpsimd
                    dq.dma_start(outr[ti], ot[:])
                elif deferred:
                    for oc in range(2):
                        poc = poolY.tile([P, QTL], F32, tag="y", name="poc")
                        nc.tensor.matmul(
                            poc[:], lhsT=yt[:, cs],
                            rhs=wp_sb[:, oc * 512:(oc + 1) * 512],
                            start=True, stop=True)
                        nc.vector.tensor_copy(
                            ot[:, oc * 512:(oc + 1) * 512], poc[:])
                    nc.gpsimd.dma_start(outr[ti], ot[:])
                else:
                    pos = poolS.tile([P, E], F32, tag="s", name="po")
                    for oc in range(2):
                        nc.tensor.matmul(
                            pos[:, oc * 512:(oc + 1) * 512],
                            lhsT=yt[:, cs],
                            rhs=wp_sb[:, oc * 512:(oc + 1) * 512],
                            start=True, stop=True)
                    nc.vector.tensor_copy(ot[:], pos[:])
                    nc.gpsimd.dma_start(outr[ti], ot[:])
                yield

        # ---- software-pipelined emission driver ----
        from collections import deque
        pending = deque()
        backlog = [0]
        rr = [0]
        reserve = []

        def pump(n):
            # round-robin across active attention generators so a finishing
            # q-tile's normalize tail interleaves with the next q-tile's
            # S blocks in every engine FIFO
            while n > 0 and pending:
                idx = rr[0] % len(pending)
                g = pending[idx]
                try:
                    next(g)
                    backlog[0] -= 1
                    n -= 1
                    rr[0] = idx + 1
                except StopIteration:
                    pending.remove(g)

        QU = 14  # units per qkv super (1 issue + 8 chunks + 1 drain + 4 v)
        for s in range(NS):
            # pace the attention backlog evenly across this super's qkv units
            # so the ScalarE exp stream never starves at a super boundary
            # keep a couple of attention units in reserve so the PE engine
            # queue never runs dry at the super boundary (the reserve drains
            # while the new super's q/k chains wait on their x DMA / weights).
            # Front-load the pumping into the q/k chunk phase (units 1..10):
            # the attention tail's DVE chain (zs copies, recip, muls) then
            # executes while the PE runs the independent q/k chains, instead
            # of queueing in front of the v-adds that gate the next super's
            # PSUM slots.
            start_backlog = max(0, backlog[0] - 2)
            done, k = 0, 0
            for u in qkv_units(s):
                k += 1
                want = min(start_backlog, (start_backlog * k + 11) // 12)
                while done < want and pending:
                    pump(1)
                    done += 1
            b, qi = divmod(s, NQT)
            pending.append(attn_units(b, qi))
            backlog[0] += (4 * qi + 4) + 6
        while pending or projq:
            while projq:
                pending.append(projq.pop(0))
            pump(1)
        while reserve:
            try:
                next(reserve[0])
            except StopIteration:
                reserve.pop(0)


_NC_CACHE = None


def _build():
    global _NC_CACHE
    if _NC_CACHE is None:
        nc = bacc.Bacc("TRN2", target_bir_lowering=False, debug=False)
        with tile.TileContext(nc) as tc:
            with ExitStack() as ctx:
                _emit(nc, tc, ctx)
        nc.compile()
        _NC_CACHE = nc
    return _NC_CACHE


def make_in_maps(x, w_qkv, b_qkv, w_proj):
    x2 = np.asarray(x, dtype=np.float32).reshape(TOK, E).astype(np.float16)
    xT = np.ascontiguousarray(x2.T)  # [E, TOK] feature-major
    w_qkv = np.asarray(w_qkv, dtype=np.float32)
    b_qkv = np.asarray(b_qkv, dtype=np.float32)
    w_proj = np.asarray(w_proj, dtype=np.float32)
    def cm(w):  # [E, P] slice -> [P, E] chunk-major fp16
        return np.ascontiguousarray(
            w.astype(np.float16).reshape(KCH, P, P).transpose(1, 0, 2)
            .reshape(P, E))

    in_maps = []
    for c in range(N_CORES):
        lo = P * c
        in_maps.append({
            "xT": xT,
            "wq": cm(w_qkv[:, lo:lo + P]),
            "wk": cm(w_qkv[:, E + lo:E + lo + P]),
            "wv": cm(w_qkv[:, 2 * E + lo:2 * E + lo + P]),
            "bqk": np.ascontiguousarray(
                np.stack([b_qkv[lo:lo + P],
                          b_qkv[E + lo:E + lo + P]], axis=1)
                .astype(np.float32)),
            "bv": np.ascontiguousarray(
                b_qkv[2 * E + lo:2 * E + lo + P].astype(np.float16)
                .reshape(1, P)),
            "wp": np.ascontiguousarray(w_proj[lo:lo + P, :].astype(np.float16)),
        })
    return in_maps


def run_sharded(inputs, trace=False, **kw):
    nc = _build()
    in_maps = make_in_maps(inputs["x"], inputs["w_qkv"], inputs["b_qkv"],
                           inputs["w_proj"])
    res = run_bass_kernel_spmd(nc, in_maps, list(range(N_CORES)), trace=trace,
                               **kw)
    partial = np.zeros((TOK, E), dtype=np.float32)
    for i in range(N_CORES):
        partial += res.results[i]["out"]
    out = partial + np.asarray(inputs["b_proj"], dtype=np.float32)[None, :]
    return out.reshape(B, T, E), res


def kernel(**inputs) -> np.ndarray:
    out, _ = run_sharded(inputs, trace=False)
    return out



# revision 66
# speedup vs baseline: 1.0129x; 1.0129x over previous
"""Trainium2 Bass kernel: 16-head causal self-attention block (QKV proj ->
causal MHA -> output proj), tensor-parallel over heads across 8 NeuronCores.

Contract: kernel(**inputs) takes FULL unsharded inputs
  x      [2, 2048, 1024] f32
  w_qkv  [1024, 3072] f32, b_qkv [3072] f32
  w_proj [1024, 1024] f32, b_proj [1024] f32
and returns the FULL output [2, 2048, 1024] f32.

Sharding: head-parallel. Core c owns global heads (2c, 2c+1):
  - column-parallel QKV (each core takes its 128 q/k/v feature columns)
  - full causal attention for its 2 heads (both batches)
  - row-parallel output projection -> partial [4096, 1024] sums
  - host reduces the 8 partials and adds b_proj.

v2 dataflow (vs v1): software-pipelined emission interleaves QKV of
super-tile s with attention of super s-1 so the PE never starves while
ScalarE runs softmax exp (ScalarE does ONLY exp + a few psum drains; no
Ln -> no act-table thrash). q/k/P/v all fp16 on the PE. v is computed in
natural [token, feat] orientation directly (xT stationary), killing the
PE transposes; v bias folds in at the PSUM drain via a precomputed
broadcast tile. x is pre-transposed on the host so all x loads are plain
contiguous DMAs.

v3 (vs v2, 217.7us -> ~168.5us):
- 1/Z via single-op DVE reciprocal_approx_fast on an SBUF bounce of the
  replicated Z ones-rows of the AV PSUM accumulator (drops the
  Z-broadcast matmul and 30us of exact DVE reciprocal from the
  normalize critical path; the custom op cannot read PSUM directly).
- q/k bias drains moved to ScalarE activation(Identity, bias=AP): DVE
  decongests and the next attention tile's S can start sooner.
- per-tt v PSUM tiles double-buffer the v chains; causal mask-muls on
  the Pool engine (SBUF-only op); all proj drains on DVE.
- output partials drain as fp16 (halves the out-DMA that otherwise
  piles up at the tail).
- tail: supers 4,5 proj deferred into the exp-paced tail via poolY
  chunk drains; super 6's proj is a reserve pulled inline by the last
  q-tile, which pipelines its normalize+proj in 256-col halves with
  Z copies on the idle ScalarE and DMA spread over two queues. This
  keeps the PE dense enough that the HAM clock holds k=8 until the
  final matmul retires.
"""

import numpy as np
from contextlib import ExitStack

import concourse.bass as bass
import concourse.tile as tile
from concourse import bacc, mybir
from concourse.bass_utils import run_bass_kernel_spmd
from concourse.masks import make_upper_triangular

F32 = mybir.dt.float32
F32R = mybir.dt.float32r
F16 = mybir.dt.float16
AF = mybir.ActivationFunctionType

N_CORES = 8
B, T, E, H, D = 2, 2048, 1024, 16, 64
TOK = B * T          # 4096 tokens
P = 128              # partitions
SUPER = 512          # tokens per QKV super-tile
NS = TOK // SUPER    # 8 super-tiles
KCH = E // P         # 8 contraction chunks
QTL = 512            # attention q-tile width
NQT = T // QTL       # 4 q-tiles per batch
KBL = 128            # attention k-block height
VAW = 256            # v_aug cols per token tile: 2 heads x (64 v + 64 ones);
                     # the 64 replicated ones-columns make the AV matmul emit
                     # the softmax denominator Z broadcast on partitions 64-127


def r(ap):
    return ap.bitcast(F32R)


def _emit(nc, tc, ctx):
    # weights arrive host-relayouted: [P, E] chunk-major, one contiguous DMA
    xT_h = nc.declare_dram_parameter("xT", [E, TOK], F16, isOutput=False)
    wq_h = nc.declare_dram_parameter("wq", [P, E], F16, isOutput=False)
    wk_h = nc.declare_dram_parameter("wk", [P, E], F16, isOutput=False)
    wv_h = nc.declare_dram_parameter("wv", [P, E], F16, isOutput=False)
    bqk_h = nc.declare_dram_parameter("bqk", [P, 2], F32, isOutput=False)
    bv_h = nc.declare_dram_parameter("bv", [1, P], F16, isOutput=False)
    wp_h = nc.declare_dram_parameter("wp", [P, E], F16, isOutput=False)
    out_h = nc.declare_dram_parameter("out", [TOK, E], F16, isOutput=True)

    outr = out_h[:].rearrange("(n p) e -> n p e", p=P)  # [32, 128, 1024]

    # ---------------- persistent tiles ----------------
    const = ctx.enter_context(tc.tile_pool(name="const", bufs=1))
    mask_tri = const.tile([P, P], F16)  # mask[p, f] = 1.0 iff p <= f
    make_upper_triangular(nc, mask_tri[:], val=1.0, diag=True)
    ones1h = const.tile([1, P], F16)
    nc.vector.memset(ones1h[:], 1.0)


    # weight/bias loads go on the Activation queue (idle at startup) so the
    # x loads on the sync queue aren't delayed behind them
    bqk_sb = const.tile([P, 2], F32)
    bvr = const.tile([1, P], F16)
    nc.scalar.dma_start(bqk_sb[:], bqk_h[:])
    nc.scalar.dma_start(bvr[:], bv_h[:])
    bq_sb = bqk_sb[:, 0:1]
    bk_sb = bqk_sb[:, 1:2]
    wq_sb = const.tile([P, E], F16)
    wk_sb = const.tile([P, E], F16)
    wv_sb = const.tile([P, E], F16)
    wp_sb = const.tile([P, E], F16)
    # one weight per queue: a single queue moves these serially at ~100GB/s
    # and the first q-chain would wait ~10us for wq otherwise
    for wsb, wh in ((wq_sb, wq_h), (wk_sb, wk_h), (wv_sb, wv_h),
                    (wp_sb, wp_h)):
        nc.scalar.dma_start(wsb[:], wh[:])
    bvb = const.tile([P, P], F32)  # v bias broadcast to all 128 partitions

    persist = ctx.enter_context(tc.tile_pool(name="persist", bufs=1))

    with ExitStack() as ph:
        xpool = ph.enter_context(tc.tile_pool(name="xp", bufs=4))
        ptpool = ph.enter_context(tc.tile_pool(name="pTp", bufs=12))
        zbpool = ph.enter_context(tc.tile_pool(name="zbp", bufs=2))
        opool = ph.enter_context(tc.tile_pool(name="op", bufs=6))
        poolQ = ph.enter_context(tc.tile_pool(name="poolQ", bufs=2, space="PSUM"))
        poolS = ph.enter_context(tc.tile_pool(name="poolS", bufs=2, space="PSUM"))
        poolY = ph.enter_context(tc.tile_pool(name="poolY", bufs=2, space="PSUM"))

        # v bias broadcast: bvb[p, j] = bv[j] via rank-1 matmul of ones x bv
        pb = poolQ.tile([P, P], F32, tag="q", name="pb")
        # HAM warmup: the PE would otherwise idle ~6us waiting for the weight
        # DMAs and start the real QKV chains at half clock (K=4/8). A burst
        # of dependency-free matmuls (ones x ones, overwritten below) keeps
        # the PE activity window busy so the clock-gate opens first.
        for _ in range(48):
            nc.tensor.matmul(pb[0:32, :], lhsT=ones1h[0:1, 0:32],
                             rhs=ones1h[:], start=True, stop=True)
        nc.tensor.matmul(pb[:], lhsT=ones1h[:], rhs=bvr[:], start=True,
                         stop=True)
        nc.vector.tensor_copy(bvb[:], pb[:])

        yts = [persist.tile([P, QTL], F16, tag=f"yt{n}", name="yt")
               for n in range(NS)]
        projq = []
        qTs, kTs, vas = [], [], []
        for s in range(NS):
            qTs.append(persist.tile([P, SUPER], F16, tag=f"qT{s}", name="qTt"))
            kTs.append(persist.tile([P, SUPER], F16, tag=f"kT{s}", name="kTt"))
            vat = persist.tile([P, 4 * VAW], F16, tag=f"va{s}", name="vat")
            vas.append(vat)
            # whole tile starts as ones; the per-super v drains overwrite the
            # v columns, leaving the replicated ones-columns (64-127 of each
            # head group) that make the AV matmul emit Z on partitions 64-127
            nc.gpsimd.memset(vat[:], 1.0)

        xtiles = {}
        xT_cm = xT_h[:].rearrange("(c p) tok -> p c tok", p=P)  # [128,8,4096]

        def issue_x(s):
            # one strided DMA per super-tile; chunk ch lands contiguous at
            # cols [ch*512, (ch+1)*512). The first super is split in half so
            # its q-chain can start on chunks 0-3 while 4-7 are in flight.
            xt = xpool.tile([P, KCH * SUPER], F16, tag="xT", name="xTt")
            nc.sync.dma_start(
                xt[:].rearrange("p (c t) -> p c t", c=KCH),
                xT_cm[:, :, s * SUPER:(s + 1) * SUPER])
            xtiles[s] = [xt[:, ch * SUPER:(ch + 1) * SUPER]
                         for ch in range(KCH)]

        issue_x(0)
        issue_x(1)
        issue_x(2)

        def qkv_units(s):
            xt = xtiles[s]
            if s + 3 < NS:
                issue_x(s + 3)
            yield
            # q chain fully before k chain: pfq stops ~2us earlier, so its
            # ScalarE drain overlaps the k chain and the poolQ slot is
            # already free when the v chains (and the next super's q) need
            # it -- this was the recurring super-boundary PE stall
            pfq = poolQ.tile([P, SUPER], F32, tag="q", name="pfq")
            for ch in range(KCH):
                nc.tensor.matmul(
                    pfq[:], lhsT=wq_sb[:, ch * P:(ch + 1) * P],
                    rhs=xt[ch][:], start=(ch == 0), stop=(ch == KCH - 1))
                if ch % 2 == 1:
                    yield
            # drain on ScalarE (bias folds into the activation) so the
            # attention S-start never queues behind DVE
            nc.scalar.activation(qTs[s][:], pfq[:], AF.Identity, bias=bq_sb)
            pfk = poolQ.tile([P, SUPER], F32, tag="q", name="pfk")
            for ch in range(KCH):
                nc.tensor.matmul(
                    pfk[:], lhsT=wk_sb[:, ch * P:(ch + 1) * P],
                    rhs=xt[ch][:], start=(ch == 0), stop=(ch == KCH - 1))
                if ch % 2 == 1:
                    yield
            nc.scalar.activation(kTs[s][:], pfk[:], AF.Identity, bias=bk_sb)
            yield
            bvb2 = bvb[:].rearrange("p (h d) -> p h d", h=2)
            for tt in range(4):
                vps = poolQ.tile([P, P], F32, tag="q", name="vps")
                for ch in range(KCH):
                    nc.tensor.matmul(
                        vps[:],
                        lhsT=xt[ch][:, tt * P:(tt + 1) * P],
                        rhs=wv_sb[:, ch * P:(ch + 1) * P],
                        start=(ch == 0), stop=(ch == KCH - 1))
                dst = vas[s][:, tt * VAW:(tt + 1) * VAW].rearrange(
                    "p (h x) -> p h x", x=2 * D)[:, :, 0:D]
                src = vps[:].rearrange("p (h d) -> p h d", h=2)
                nc.vector.tensor_add(dst, src, bvb2)
                yield

        projq = []

        def attn_units(b, qi):
            nkb = 4 * qi + 4   # k blocks of 128 covering [0, (qi+1)*512)
            sq = 4 * b + qi    # super-tile holding this q range
            # the last q-tile takes its accumulators from poolQ (idle once
            # QKV is done) so its blocks overlap the previous q-tile's tail
            ypool, ytag = (poolQ, "q") if sq == NS - 1 else (poolY, "y")
            pys = [ypool.tile([P, QTL], F32, tag=ytag, name=f"py{h}")
                   for h in range(2)]

            def emit_S(kb):
                c0 = max(0, kb * KBL - qi * QTL)
                sk, kc = 4 * b + kb // 4, (kb % 4) * KBL
                ps = poolS.tile([P, 2 * QTL], F32, tag="s", name="ps")
                for h in range(2):
                    nc.tensor.matmul(
                        ps[:, h * QTL + c0:(h + 1) * QTL],
                        lhsT=kTs[sk][64 * h:64 * h + 64, kc:kc + KBL],
                        rhs=qTs[sq][64 * h:64 * h + 64, c0:QTL],
                        start=True, stop=True)
                return ps, c0

            cur = emit_S(0)
            for kb in range(nkb):
                ps, c0 = cur
                if kb + 1 < nkb:
                    cur = emit_S(kb + 1)
                pt = ptpool.tile([P, 2 * QTL], F16, tag="pT", name="pt")
                if c0 == 0:
                    nc.scalar.activation(pt[:], ps[:], AF.Exp, scale=0.125)
                else:
                    src = ps[:].rearrange("p (h q) -> p h q", h=2)[:, :, c0:]
                    dst = pt[:].rearrange("p (h q) -> p h q", h=2)[:, :, c0:]
                    nc.scalar.activation(dst, src, AF.Exp, scale=0.125)
                if kb * KBL >= qi * QTL:  # diagonal block: causal triangle
                    sl = pt[:].rearrange("p (h q) -> p h q",
                                         h=2)[:, :, c0:c0 + P]
                    m3 = mask_tri[:].rearrange(
                        "p (u f) -> p u f", u=1).broadcast_to([P, 2, P])
                    # SBUF-only op: run it on the (otherwise idle) Pool
                    # engine so DVE keeps its bandwidth for PSUM drains
                    nc.gpsimd.tensor_mul(sl, sl, m3)
                vo = (kb % 4) * VAW
                sk = 4 * b + kb // 4
                for h in range(2):
                    nc.tensor.matmul(
                        pys[h][:, c0:QTL],
                        lhsT=vas[sk][:, vo + 2 * D * h:vo + 2 * D * h + 2 * D],
                        rhs=pt[:, h * QTL + c0:(h + 1) * QTL],
                        start=(kb == 0), stop=(kb == nkb - 1))
                # keep-warm: a dependency-free weight load keeps the PE's
                # HAM activity window busy through exp-wait micro-idles
                nc.tensor.ldweights(weights=mask_tri[0:1, 0:32])
                yield
            # normalize: y * (1/Z). pys[h] partitions 64-127 already hold Z
            # broadcast (the replicated ones-columns), so a fast approximate
            # reciprocal reads them straight out of PSUM -- no Z-broadcast
            # matmul, no exact (slow) reciprocal on the critical path. The
            # custom-DVE recip needs raw fp32 bits and PSUM reads break its
            # BITWISE_NOT seed, so bounce Z through SBUF first.
            yt = yts[sq]
            if sq == NS - 1:
                # last q-tile: nothing overlaps its tail, so pipeline
                # normalize+proj in 256-col halves to keep the PE fed and
                # the final out-DMA issued as early as possible. PE filler
                # during the Z-drain latency comes from the reserve (the
                # last deferred super's proj units); the Z copies go on
                # ScalarE, which is idle once the final exp retires.
                def pull(n):
                    while n > 0 and reserve:
                        try:
                            next(reserve[0])
                            n -= 1
                        except StopIteration:
                            reserve.pop(0)
                pull(3)
                for half in range(2):
                    hs = slice(half * 2 * P, (half + 1) * 2 * P)
                    zs = zbpool.tile([P, 2 * P], F32, tag="zb", name="zs")
                    # one Z copy per engine so they run in parallel
                    nc.vector.tensor_copy(zs[0:D, :], pys[0][D:2 * D, hs])
                    nc.scalar.activation(zs[D:2 * D, :], pys[1][D:2 * D, hs],
                                         AF.Copy)
                    zinv = zbpool.tile([P, 2 * P], F32, tag="zb", name="zinv")
                    nc.vector.reciprocal_approx_fast(zinv[:], zs[:])
                    nc.vector.tensor_mul(yt[0:D, hs], pys[0][0:D, hs],
                                         zinv[0:D, :])
                    nc.vector.tensor_mul(yt[D:2 * D, hs], pys[1][0:D, hs],
                                         zinv[D:2 * D, :])
                    pull(1)
                    yield
                    for tt4 in (2 * half, 2 * half + 1):
                        cs = slice(tt4 * P, (tt4 + 1) * P)
                        pos = poolS.tile([P, E], F32, tag="s", name="po")
                        for oc in range(2):
                            nc.tensor.matmul(
                                pos[:, oc * 512:(oc + 1) * 512],
                                lhsT=yt[:, cs],
                                rhs=wp_sb[:, oc * 512:(oc + 1) * 512],
                                start=True, stop=True)
                        ti = (b * T + qi * QTL) // P + tt4
                        ot = opool.tile([P, E], F16, tag="ot", name="ot")
                        # drain halves on DVE + ScalarE (idle after the
                        # final exp) in parallel, and DMA each half as soon
                        # as its drain lands, on separate queues -- the
                        # kernel cannot retire until the last byte is out
                        nc.vector.tensor_copy(ot[:, 0:512], pos[:, 0:512])
                        nc.scalar.activation(ot[:, 512:1024],
                                             pos[:, 512:1024], AF.Copy)
                        nc.gpsimd.dma_start(outr[ti][:, 0:512],
                                            ot[:, 0:512])
                        nc.sync.dma_start(outr[ti][:, 512:1024],
                                          ot[:, 512:1024])
                        yield
                return
            zs = zbpool.tile([P, QTL], F32, tag="zb", name="zs")
            nc.vector.tensor_copy(zs[0:D, :], pys[0][D:2 * D, :])
            nc.vector.tensor_copy(zs[D:2 * D, :], pys[1][D:2 * D, :])
            zinv = zbpool.tile([P, QTL], F32, tag="zb", name="zinv")
            nc.vector.reciprocal_approx_fast(zinv[:], zs[:])
            yield
            nc.vector.tensor_mul(yt[0:D, :], pys[0][0:D, :], zinv[0:D, :])
            nc.vector.tensor_mul(yt[D:2 * D, :], pys[1][0:D, :],
                                 zinv[D:2 * D, :])
            yield
            if sq in (4, 5):
                # deferred: dense PE work reserved for the exp-paced tail so
                # the HAM clock stays warm through the last q-tiles
                projq.append(proj_units(b, qi))
            elif sq == 6:
                # super 6's proj is the reserve the last q-tile pulls from
                reserve.append(proj_units(b, qi, tail=True))
            else:
                yield from proj_units(b, qi)

        def proj_units(b, qi, tail=False):
            yt = yts[4 * b + qi]
            # mid-tail-deferred tiles take their PSUM from poolY (idle in
            # the tail: the last q-tile's accumulators live in poolQ), so
            # the tail's S-block double-buffer in poolS never blocks on a
            # proj drain. The reserve tiles pulled during the last q-tile's
            # normalize use poolS (S-blocks are done by then) and drain in
            # halves on DVE+ScalarE so they never queue in front of the
            # normalize chain on DVE.
            deferred = 4 * b + qi in (4, 5)
            for tt4 in range(4):
                cs = slice(tt4 * P, (tt4 + 1) * P)
                ti = (b * T + qi * QTL) // P + tt4
                ot = opool.tile([P, E], F16, tag="ot", name="ot")
                if tail:
                    pos = poolS.tile([P, E], F32, tag="s", name="po")
                    for oc in range(2):
                        nc.tensor.matmul(
                            pos[:, oc * 512:(oc + 1) * 512],
                            lhsT=yt[:, cs],
                            rhs=wp_sb[:, oc * 512:(oc + 1) * 512],
                            start=True, stop=True)
                    nc.vector.tensor_copy(ot[:, 0:512], pos[:, 0:512])
                    nc.scalar.activation(ot[:, 512:1024], pos[:, 512:1024],
                                         AF.Copy)
                    dq = nc.sync if tt4 % 2 else nc.gpsimd
                    dq.dma_start(outr[ti], ot[:])
                elif deferred:
                    for oc in range(2):
                        poc = poolY.tile([P, QTL], F32, tag="y", name="poc")
                        nc.tensor.matmul(
                            poc[:], lhsT=yt[:, cs],
                            rhs=wp_sb[:, oc * 512:(oc + 1) * 512],
                            start=True, stop=True)
                        nc.vector.tensor_copy(
                            ot[:, oc * 512:(oc + 1) * 512], poc[:])
                    nc.gpsimd.dma_start(outr[ti], ot[:])
                else:
                    pos = poolS.tile([P, E], F32, tag="s", name="po")
                    for oc in range(2):
                        nc.tensor.matmul(
                            pos[:, oc * 512:(oc + 1) * 512],
                            lhsT=yt[:, cs],
                            rhs=wp_sb[:, oc * 512:(oc + 1) * 512],
                            start=True, stop=True)
                    nc.vector.tensor_copy(ot[:], pos[:])
                    nc.gpsimd.dma_start(outr[ti], ot[:])
                yield

        # ---- software-pipelined emission driver ----
        from collections import deque
        pending = deque()
        backlog = [0]
        rr = [0]
        reserve = []

        def pump(n):
            # round-robin across active attention generators so a finishing
            # q-tile's normalize tail interleaves with the next q-tile's
            # S blocks in every engine FIFO
            while n > 0 and pending:
                idx = rr[0] % len(pending)
                g = pending[idx]
                try:
                    next(g)
                    backlog[0] -= 1
                    n -= 1
                    rr[0] = idx + 1
                except StopIteration:
                    pending.remove(g)

        QU = 14  # units per qkv super (1 issue + 8 chunks + 1 drain + 4 v)
        for s in range(NS):
            # pace the attention backlog evenly across this super's qkv units
            # so the ScalarE exp stream never starves at a super boundary
            # keep a couple of attention units in reserve so the PE engine
            # queue never runs dry at the super boundary (the reserve drains
            # while the new super's q/k chains wait on their x DMA / weights).
            # Front-load the pumping into the q/k chunk phase (units 1..10):
            # the attention tail's DVE chain (zs copies, recip, muls) then
            # executes while the PE runs the independent q/k chains, instead
            # of queueing in front of the v-adds that gate the next super's
            # PSUM slots.
            start_backlog = max(0, backlog[0] - 2)
            done, k = 0, 0
            for u in qkv_units(s):
                k += 1
                want = min(start_backlog, (start_backlog * k + 11) // 12)
                while done < want and pending:
                    pump(1)
                    done += 1
            b, qi = divmod(s, NQT)
            pending.append(attn_units(b, qi))
            backlog[0] += (4 * qi + 4) + 6
        while pending or projq:
            while projq:
                pending.append(projq.pop(0))
            pump(1)
        while reserve:
            try:
                next(reserve[0])
            except StopIteration:
                reserve.pop(0)


_NC_CACHE = None


def _build():
    global _NC_CACHE
    if _NC_CACHE is None:
        nc = bacc.Bacc("TRN2", target_bir_lowering=False, debug=False)
        with tile.TileContext(nc) as tc:
            with ExitStack() as ctx:
                _emit(nc, tc, ctx)
        nc.compile()
        _NC_CACHE = nc
    return _NC_CACHE


def make_in_maps(x, w_qkv, b_qkv, w_proj):
    x2 = np.asarray(x, dtype=np.float32).reshape(TOK, E).astype(np.float16)
    xT = np.ascontiguousarray(x2.T)  # [E, TOK] feature-major
    w_qkv = np.asarray(w_qkv, dtype=np.float32)
    b_qkv = np.asarray(b_qkv, dtype=np.float32)
    w_proj = np.asarray(w_proj, dtype=np.float32)
    def cm(w):  # [E, P] slice -> [P, E] chunk-major fp16
        return np.ascontiguousarray(
            w.astype(np.float16).reshape(KCH, P, P).transpose(1, 0, 2)
            .reshape(P, E))

    in_maps = []
    for c in range(N_CORES):
        lo = P * c
        in_maps.append({
            "xT": xT,
            "wq": cm(w_qkv[:, lo:lo + P]),
            "wk": cm(w_qkv[:, E + lo:E + lo + P]),
            "wv": cm(w_qkv[:, 2 * E + lo:2 * E + lo + P]),
            "bqk": np.ascontiguousarray(
                np.stack([b_qkv[lo:lo + P],
                          b_qkv[E + lo:E + lo + P]], axis=1)
                .astype(np.float32)),
            "bv": np.ascontiguousarray(
                b_qkv[2 * E + lo:2 * E + lo + P].astype(np.float16)
                .reshape(1, P)),
            "wp": np.ascontiguousarray(w_proj[lo:lo + P, :].astype(np.float16)),
        })
    return in_maps


def run_sharded(inputs, trace=False, **kw):
    nc = _build()
    in_maps = make_in_maps(inputs["x"], inputs["w_qkv"], inputs["b_qkv"],
                           inputs["w_proj"])
    res = run_bass_kernel_spmd(nc, in_maps, list(range(N_CORES)), trace=trace,
                               **kw)
    partial = np.zeros((TOK, E), dtype=np.float32)
    for i in range(N_CORES):
        partial += res.results[i]["out"]
    out = partial + np.asarray(inputs["b_proj"], dtype=np.float32)[None, :]
    return out.reshape(B, T, E), res


def kernel(**inputs) -> np.ndarray:
    out, _ = run_sharded(inputs, trace=False)
    return out



# revision 67
# speedup vs baseline: 1.0387x; 1.0255x over previous
"""Trainium2 Bass kernel: 16-head causal self-attention block (QKV proj ->
causal MHA -> output proj), tensor-parallel over heads across 8 NeuronCores.

Contract: kernel(**inputs) takes FULL unsharded inputs
  x      [2, 2048, 1024] f32
  w_qkv  [1024, 3072] f32, b_qkv [3072] f32
  w_proj [1024, 1024] f32, b_proj [1024] f32
and returns the FULL output [2, 2048, 1024] f32.

Sharding: head-parallel. Core c owns global heads (2c, 2c+1):
  - column-parallel QKV (each core takes its 128 q/k/v feature columns)
  - full causal attention for its 2 heads (both batches)
  - row-parallel output projection -> partial [4096, 1024] sums
  - host reduces the 8 partials and adds b_proj.

v2 dataflow (vs v1): software-pipelined emission interleaves QKV of
super-tile s with attention of super s-1 so the PE never starves while
ScalarE runs softmax exp (ScalarE does ONLY exp + a few psum drains; no
Ln -> no act-table thrash). q/k/P/v all fp16 on the PE. v is computed in
natural [token, feat] orientation directly (xT stationary), killing the
PE transposes; v bias folds in at the PSUM drain via a precomputed
broadcast tile. x is pre-transposed on the host so all x loads are plain
contiguous DMAs.

v3 (vs v2, 217.7us -> ~168.5us):
- 1/Z via single-op DVE reciprocal_approx_fast on an SBUF bounce of the
  replicated Z ones-rows of the AV PSUM accumulator (drops the
  Z-broadcast matmul and 30us of exact DVE reciprocal from the
  normalize critical path; the custom op cannot read PSUM directly).
- q/k bias drains moved to ScalarE activation(Identity, bias=AP): DVE
  decongests and the next attention tile's S can start sooner.
- per-tt v PSUM tiles double-buffer the v chains; causal mask-muls on
  the Pool engine (SBUF-only op); all proj drains on DVE.
- output partials drain as fp16 (halves the out-DMA that otherwise
  piles up at the tail).
- tail: supers 4,5 proj deferred into the exp-paced tail via poolY
  chunk drains; super 6's proj is a reserve pulled inline by the last
  q-tile, which pipelines its normalize+proj in 256-col halves with
  Z copies on the idle ScalarE and DMA spread over two queues. This
  keeps the PE dense enough that the HAM clock holds k=8 until the
  final matmul retires.
"""

import numpy as np
from contextlib import ExitStack

import concourse.bass as bass
import concourse.tile as tile
from concourse import bacc, mybir
from concourse.bass_utils import run_bass_kernel_spmd
from concourse.masks import make_upper_triangular

F32 = mybir.dt.float32
F32R = mybir.dt.float32r
F16 = mybir.dt.float16
AF = mybir.ActivationFunctionType

N_CORES = 8
B, T, E, H, D = 2, 2048, 1024, 16, 64
TOK = B * T          # 4096 tokens
P = 128              # partitions
SUPER = 512          # tokens per QKV super-tile
NS = TOK // SUPER    # 8 super-tiles
KCH = E // P         # 8 contraction chunks
QTL = 512            # attention q-tile width
NQT = T // QTL       # 4 q-tiles per batch
KBL = 128            # attention k-block height
VAW = 256            # v_aug cols per token tile: 2 heads x (64 v + 64 ones);
                     # the 64 replicated ones-columns make the AV matmul emit
                     # the softmax denominator Z broadcast on partitions 64-127


def r(ap):
    return ap.bitcast(F32R)


def _emit(nc, tc, ctx):
    # weights arrive host-relayouted: [P, E] chunk-major, one contiguous DMA
    xT_h = nc.declare_dram_parameter("xT", [E, TOK], F16, isOutput=False)
    wq_h = nc.declare_dram_parameter("wq", [P, E], F16, isOutput=False)
    wk_h = nc.declare_dram_parameter("wk", [P, E], F16, isOutput=False)
    wv_h = nc.declare_dram_parameter("wv", [P, E], F16, isOutput=False)
    bqk_h = nc.declare_dram_parameter("bqk", [P, 2], F32, isOutput=False)
    bv_h = nc.declare_dram_parameter("bv", [1, P], F16, isOutput=False)
    wp_h = nc.declare_dram_parameter("wp", [P, E], F16, isOutput=False)
    out_h = nc.declare_dram_parameter("out", [TOK, E], F16, isOutput=True)

    outr = out_h[:].rearrange("(n p) e -> n p e", p=P)  # [32, 128, 1024]

    # ---------------- persistent tiles ----------------
    const = ctx.enter_context(tc.tile_pool(name="const", bufs=1))
    mask_tri = const.tile([P, P], F16)  # mask[p, f] = 1.0 iff p <= f
    make_upper_triangular(nc, mask_tri[:], val=1.0, diag=True)
    ones1h = const.tile([1, P], F16)
    nc.vector.memset(ones1h[:], 1.0)


    # weight/bias loads go on the Activation queue (idle at startup) so the
    # x loads on the sync queue aren't delayed behind them
    bqk_sb = const.tile([P, 2], F32)
    bvr = const.tile([1, P], F16)
    nc.scalar.dma_start(bqk_sb[:], bqk_h[:])
    nc.scalar.dma_start(bvr[:], bv_h[:])
    bq_sb = bqk_sb[:, 0:1]
    bk_sb = bqk_sb[:, 1:2]
    wq_sb = const.tile([P, E], F16)
    wk_sb = const.tile([P, E], F16)
    wv_sb = const.tile([P, E], F16)
    wp_sb = const.tile([P, E], F16)
    # one weight per queue: a single queue moves these serially at ~100GB/s
    # and the first q-chain would wait ~10us for wq otherwise
    for wsb, wh in ((wq_sb, wq_h), (wk_sb, wk_h), (wv_sb, wv_h),
                    (wp_sb, wp_h)):
        nc.scalar.dma_start(wsb[:], wh[:])
    bvb = const.tile([P, P], F32)  # v bias broadcast to all 128 partitions

    persist = ctx.enter_context(tc.tile_pool(name="persist", bufs=1))

    with ExitStack() as ph:
        xpool = ph.enter_context(tc.tile_pool(name="xp", bufs=4))
        ptpool = ph.enter_context(tc.tile_pool(name="pTp", bufs=12))
        zbpool = ph.enter_context(tc.tile_pool(name="zbp", bufs=2))
        opool = ph.enter_context(tc.tile_pool(name="op", bufs=6))
        poolQ = ph.enter_context(tc.tile_pool(name="poolQ", bufs=2, space="PSUM"))
        poolS = ph.enter_context(tc.tile_pool(name="poolS", bufs=2, space="PSUM"))
        poolY = ph.enter_context(tc.tile_pool(name="poolY", bufs=2, space="PSUM"))

        # v bias broadcast: bvb[p, j] = bv[j] via rank-1 matmul of ones x bv
        pb = poolQ.tile([P, P], F32, tag="q", name="pb")
        # HAM warmup: the PE would otherwise idle ~6us waiting for the weight
        # DMAs and start the real QKV chains at half clock (K=4/8). A burst
        # of dependency-free matmuls (ones x ones, overwritten below) keeps
        # the PE activity window busy so the clock-gate opens first.
        for _ in range(48):
            nc.tensor.matmul(pb[0:32, :], lhsT=ones1h[0:1, 0:32],
                             rhs=ones1h[:], start=True, stop=True)
        nc.tensor.matmul(pb[:], lhsT=ones1h[:], rhs=bvr[:], start=True,
                         stop=True)
        nc.vector.tensor_copy(bvb[:], pb[:])

        yts = [persist.tile([P, QTL], F16, tag=f"yt{n}", name="yt")
               for n in range(NS)]
        projq = []
        qTs, kTs, vas = [], [], []
        for s in range(NS):
            qTs.append(persist.tile([P, SUPER], F16, tag=f"qT{s}", name="qTt"))
            kTs.append(persist.tile([P, SUPER], F16, tag=f"kT{s}", name="kTt"))
            vat = persist.tile([P, 4 * VAW], F16, tag=f"va{s}", name="vat")
            vas.append(vat)
            # whole tile starts as ones; the per-super v drains overwrite the
            # v columns, leaving the replicated ones-columns (64-127 of each
            # head group) that make the AV matmul emit Z on partitions 64-127
            nc.gpsimd.memset(vat[:], 1.0)

        xtiles = {}
        xT_cm = xT_h[:].rearrange("(c p) tok -> p c tok", p=P)  # [128,8,4096]

        def issue_x(s):
            # one strided DMA per super-tile; chunk ch lands contiguous at
            # cols [ch*512, (ch+1)*512). The first super is split in half so
            # its q-chain can start on chunks 0-3 while 4-7 are in flight.
            xt = xpool.tile([P, KCH * SUPER], F16, tag="xT", name="xTt")
            nc.sync.dma_start(
                xt[:].rearrange("p (c t) -> p c t", c=KCH),
                xT_cm[:, :, s * SUPER:(s + 1) * SUPER])
            xtiles[s] = [xt[:, ch * SUPER:(ch + 1) * SUPER]
                         for ch in range(KCH)]

        issue_x(0)
        issue_x(1)
        issue_x(2)

        def qkv_units(s):
            xt = xtiles[s]
            if s + 3 < NS:
                issue_x(s + 3)
            yield
            # q chain fully before k chain: pfq stops ~2us earlier, so its
            # ScalarE drain overlaps the k chain and the poolQ slot is
            # already free when the v chains (and the next super's q) need
            # it -- this was the recurring super-boundary PE stall
            pfq = poolQ.tile([P, SUPER], F32, tag="q", name="pfq")
            for ch in range(KCH):
                nc.tensor.matmul(
                    pfq[:], lhsT=wq_sb[:, ch * P:(ch + 1) * P],
                    rhs=xt[ch][:], start=(ch == 0), stop=(ch == KCH - 1))
                if ch % 2 == 1:
                    yield
            # drain on ScalarE (bias folds into the activation) so the
            # attention S-start never queues behind DVE
            nc.scalar.activation(qTs[s][:], pfq[:], AF.Identity, bias=bq_sb)
            pfk = poolQ.tile([P, SUPER], F32, tag="q", name="pfk")
            for ch in range(KCH):
                nc.tensor.matmul(
                    pfk[:], lhsT=wk_sb[:, ch * P:(ch + 1) * P],
                    rhs=xt[ch][:], start=(ch == 0), stop=(ch == KCH - 1))
                if ch % 2 == 1:
                    yield
            nc.scalar.activation(kTs[s][:], pfk[:], AF.Identity, bias=bk_sb)
            yield
            bvb2 = bvb[:].rearrange("p (h d) -> p h d", h=2)
            for tt in range(4):
                vps = poolQ.tile([P, P], F32, tag="q", name="vps")
                for ch in range(KCH):
                    nc.tensor.matmul(
                        vps[:],
                        lhsT=xt[ch][:, tt * P:(tt + 1) * P],
                        rhs=wv_sb[:, ch * P:(ch + 1) * P],
                        start=(ch == 0), stop=(ch == KCH - 1))
                dst = vas[s][:, tt * VAW:(tt + 1) * VAW].rearrange(
                    "p (h x) -> p h x", x=2 * D)[:, :, 0:D]
                src = vps[:].rearrange("p (h d) -> p h d", h=2)
                nc.vector.tensor_add(dst, src, bvb2)
                yield

        projq = []

        def attn_units(b, qi):
            nkb = 4 * qi + 4   # k blocks of 128 covering [0, (qi+1)*512)
            sq = 4 * b + qi    # super-tile holding this q range
            # the last q-tile takes its accumulators from poolQ (idle once
            # QKV is done) so its blocks overlap the previous q-tile's tail
            ypool, ytag = (poolQ, "q") if sq == NS - 1 else (poolY, "y")
            pys = [ypool.tile([P, QTL], F32, tag=ytag, name=f"py{h}")
                   for h in range(2)]

            def emit_S(kb):
                c0 = max(0, kb * KBL - qi * QTL)
                sk, kc = 4 * b + kb // 4, (kb % 4) * KBL
                ps = poolS.tile([P, 2 * QTL], F32, tag="s", name="ps")
                for h in range(2):
                    nc.tensor.matmul(
                        ps[:, h * QTL + c0:(h + 1) * QTL],
                        lhsT=kTs[sk][64 * h:64 * h + 64, kc:kc + KBL],
                        rhs=qTs[sq][64 * h:64 * h + 64, c0:QTL],
                        start=True, stop=True)
                return ps, c0

            cur = emit_S(0)
            for kb in range(nkb):
                ps, c0 = cur
                if kb + 1 < nkb:
                    cur = emit_S(kb + 1)
                pt = ptpool.tile([P, 2 * QTL], F16, tag="pT", name="pt")
                if c0 == 0:
                    nc.scalar.activation(pt[:], ps[:], AF.Exp, scale=0.125)
                else:
                    src = ps[:].rearrange("p (h q) -> p h q", h=2)[:, :, c0:]
                    dst = pt[:].rearrange("p (h q) -> p h q", h=2)[:, :, c0:]
                    nc.scalar.activation(dst, src, AF.Exp, scale=0.125)
                if kb * KBL >= qi * QTL:  # diagonal block: causal triangle
                    sl = pt[:].rearrange("p (h q) -> p h q",
                                         h=2)[:, :, c0:c0 + P]
                    m3 = mask_tri[:].rearrange(
                        "p (u f) -> p u f", u=1).broadcast_to([P, 2, P])
                    # SBUF-only op: run it on the (otherwise idle) Pool
                    # engine so DVE keeps its bandwidth for PSUM drains
                    nc.gpsimd.tensor_mul(sl, sl, m3)
                vo = (kb % 4) * VAW
                sk = 4 * b + kb // 4
                for h in range(2):
                    nc.tensor.matmul(
                        pys[h][:, c0:QTL],
                        lhsT=vas[sk][:, vo + 2 * D * h:vo + 2 * D * h + 2 * D],
                        rhs=pt[:, h * QTL + c0:(h + 1) * QTL],
                        start=(kb == 0), stop=(kb == nkb - 1))
                yield
            # normalize: y * (1/Z). pys[h] partitions 64-127 already hold Z
            # broadcast (the replicated ones-columns), so a fast approximate
            # reciprocal reads them straight out of PSUM -- no Z-broadcast
            # matmul, no exact (slow) reciprocal on the critical path. The
            # custom-DVE recip needs raw fp32 bits and PSUM reads break its
            # BITWISE_NOT seed, so bounce Z through SBUF first.
            yt = yts[sq]
            if sq == NS - 1:
                # last q-tile: nothing overlaps its tail, so pipeline
                # normalize+proj in 256-col halves to keep the PE fed and
                # the final out-DMA issued as early as possible. PE filler
                # during the Z-drain latency comes from the reserve (the
                # last deferred super's proj units); the Z copies go on
                # ScalarE, which is idle once the final exp retires.
                def pull(n):
                    while n > 0 and reserve:
                        try:
                            next(reserve[0])
                            n -= 1
                        except StopIteration:
                            reserve.pop(0)
                pull(3)
                for half in range(2):
                    hs = slice(half * 2 * P, (half + 1) * 2 * P)
                    zs = zbpool.tile([P, 2 * P], F32, tag="zb", name="zs")
                    # one Z copy per engine so they run in parallel
                    nc.vector.tensor_copy(zs[0:D, :], pys[0][D:2 * D, hs])
                    nc.scalar.activation(zs[D:2 * D, :], pys[1][D:2 * D, hs],
                                         AF.Copy)
                    zinv = zbpool.tile([P, 2 * P], F32, tag="zb", name="zinv")
                    nc.vector.reciprocal_approx_fast(zinv[:], zs[:])
                    nc.vector.tensor_mul(yt[0:D, hs], pys[0][0:D, hs],
                                         zinv[0:D, :])
                    nc.vector.tensor_mul(yt[D:2 * D, hs], pys[1][0:D, hs],
                                         zinv[D:2 * D, :])
                    pull(1)
                    yield
                    for tt4 in (2 * half, 2 * half + 1):
                        cs = slice(tt4 * P, (tt4 + 1) * P)
                        pos = poolS.tile([P, E], F32, tag="s", name="po")
                        for oc in range(2):
                            nc.tensor.matmul(
                                pos[:, oc * 512:(oc + 1) * 512],
                                lhsT=yt[:, cs],
                                rhs=wp_sb[:, oc * 512:(oc + 1) * 512],
                                start=True, stop=True)
                        ti = (b * T + qi * QTL) // P + tt4
                        ot = opool.tile([P, E], F16, tag="ot", name="ot")
                        # drain halves on DVE + ScalarE (idle after the
                        # final exp) in parallel, and DMA each half as soon
                        # as its drain lands, on separate queues -- the
                        # kernel cannot retire until the last byte is out
                        nc.vector.tensor_copy(ot[:, 0:512], pos[:, 0:512])
                        nc.scalar.activation(ot[:, 512:1024],
                                             pos[:, 512:1024], AF.Copy)
                        nc.gpsimd.dma_start(outr[ti][:, 0:512],
                                            ot[:, 0:512])
                        nc.sync.dma_start(outr[ti][:, 512:1024],
                                          ot[:, 512:1024])
                        yield
                return
            zs = zbpool.tile([P, QTL], F32, tag="zb", name="zs")
            nc.vector.tensor_copy(zs[0:D, :], pys[0][D:2 * D, :])
            nc.vector.tensor_copy(zs[D:2 * D, :], pys[1][D:2 * D, :])
            zinv = zbpool.tile([P, QTL], F32, tag="zb", name="zinv")
            nc.vector.reciprocal_approx_fast(zinv[:], zs[:])
            yield
            nc.vector.tensor_mul(yt[0:D, :], pys[0][0:D, :], zinv[0:D, :])
            nc.vector.tensor_mul(yt[D:2 * D, :], pys[1][0:D, :],
                                 zinv[D:2 * D, :])
            yield
            if sq in (4, 5):
                # deferred: dense PE work reserved for the exp-paced tail so
                # the HAM clock stays warm through the last q-tiles
                projq.append(proj_units(b, qi))
            elif sq == 6:
                # super 6's proj is the reserve the last q-tile pulls from
                reserve.append(proj_units(b, qi, tail=True))
            else:
                yield from proj_units(b, qi)

        def proj_units(b, qi, tail=False):
            yt = yts[4 * b + qi]
            # mid-tail-deferred tiles take their PSUM from poolY (idle in
            # the tail: the last q-tile's accumulators live in poolQ), so
            # the tail's S-block double-buffer in poolS never blocks on a
            # proj drain. The reserve tiles pulled during the last q-tile's
            # normalize use poolS (S-blocks are done by then) and drain in
            # halves on DVE+ScalarE so they never queue in front of the
            # normalize chain on DVE.
            deferred = 4 * b + qi in (4, 5)
            for tt4 in range(4):
                cs = slice(tt4 * P, (tt4 + 1) * P)
                ti = (b * T + qi * QTL) // P + tt4
                ot = opool.tile([P, E], F16, tag="ot", name="ot")
                if tail:
                    pos = poolS.tile([P, E], F32, tag="s", name="po")
                    for oc in range(2):
                        nc.tensor.matmul(
                            pos[:, oc * 512:(oc + 1) * 512],
                            lhsT=yt[:, cs],
                            rhs=wp_sb[:, oc * 512:(oc + 1) * 512],
                            start=True, stop=True)
                    nc.vector.tensor_copy(ot[:, 0:512], pos[:, 0:512])
                    nc.scalar.activation(ot[:, 512:1024], pos[:, 512:1024],
                                         AF.Copy)
                    dq = nc.sync if tt4 % 2 else nc.gpsimd
                    dq.dma_start(outr[ti], ot[:])
                elif deferred:
                    for oc in range(2):
                        poc = poolY.tile([P, QTL], F32, tag="y", name="poc")
                        nc.tensor.matmul(
                            poc[:], lhsT=yt[:, cs],
                            rhs=wp_sb[:, oc * 512:(oc + 1) * 512],
                            start=True, stop=True)
                        nc.vector.tensor_copy(
                            ot[:, oc * 512:(oc + 1) * 512], poc[:])
                    nc.gpsimd.dma_start(outr[ti], ot[:])
                else:
                    pos = poolS.tile([P, E], F32, tag="s", name="po")
                    for oc in range(2):
                        nc.tensor.matmul(
                            pos[:, oc * 512:(oc + 1) * 512],
                            lhsT=yt[:, cs],
                            rhs=wp_sb[:, oc * 512:(oc + 1) * 512],
                            start=True, stop=True)
                    nc.vector.tensor_copy(ot[:], pos[:])
                    nc.gpsimd.dma_start(outr[ti], ot[:])
                yield

        # ---- software-pipelined emission driver ----
        from collections import deque
        pending = deque()
        backlog = [0]
        rr = [0]
        reserve = []

        def pump(n):
            # round-robin across active attention generators so a finishing
            # q-tile's normalize tail interleaves with the next q-tile's
            # S blocks in every engine FIFO
            while n > 0 and pending:
                idx = rr[0] % len(pending)
                g = pending[idx]
                try:
                    next(g)
                    backlog[0] -= 1
                    n -= 1
                    rr[0] = idx + 1
                except StopIteration:
                    pending.remove(g)

        QU = 14  # units per qkv super (1 issue + 8 chunks + 1 drain + 4 v)
        for s in range(NS):
            # pace the attention backlog evenly across this super's qkv units
            # so the ScalarE exp stream never starves at a super boundary
            # keep a couple of attention units in reserve so the PE engine
            # queue never runs dry at the super boundary (the reserve drains
            # while the new super's q/k chains wait on their x DMA / weights).
            # Front-load the pumping into the q/k chunk phase (units 1..10):
            # the attention tail's DVE chain (zs copies, recip, muls) then
            # executes while the PE runs the independent q/k chains, instead
            # of queueing in front of the v-adds that gate the next super's
            # PSUM slots.
            start_backlog = max(0, backlog[0] - 2)
            done, k = 0, 0
            for u in qkv_units(s):
                k += 1
                want = min(start_backlog, (start_backlog * k + 11) // 12)
                while done < want and pending:
                    pump(1)
                    done += 1
            b, qi = divmod(s, NQT)
            pending.append(attn_units(b, qi))
            backlog[0] += (4 * qi + 4) + 6
        while pending or projq:
            while projq:
                pending.append(projq.pop(0))
            pump(1)
        while reserve:
            try:
                next(reserve[0])
            except StopIteration:
                reserve.pop(0)


_NC_CACHE = None


def _build():
    global _NC_CACHE
    if _NC_CACHE is None:
        nc = bacc.Bacc("TRN2", target_bir_lowering=False, debug=False)
        with tile.TileContext(nc) as tc:
            with ExitStack() as ctx:
                _emit(nc, tc, ctx)
        nc.compile()
        _NC_CACHE = nc
    return _NC_CACHE


def make_in_maps(x, w_qkv, b_qkv, w_proj):
    x2 = np.asarray(x, dtype=np.float32).reshape(TOK, E).astype(np.float16)
    xT = np.ascontiguousarray(x2.T)  # [E, TOK] feature-major
    w_qkv = np.asarray(w_qkv, dtype=np.float32)
    b_qkv = np.asarray(b_qkv, dtype=np.float32)
    w_proj = np.asarray(w_proj, dtype=np.float32)
    def cm(w):  # [E, P] slice -> [P, E] chunk-major fp16
        return np.ascontiguousarray(
            w.astype(np.float16).reshape(KCH, P, P).transpose(1, 0, 2)
            .reshape(P, E))

    in_maps = []
    for c in range(N_CORES):
        lo = P * c
        in_maps.append({
            "xT": xT,
            "wq": cm(w_qkv[:, lo:lo + P]),
            "wk": cm(w_qkv[:, E + lo:E + lo + P]),
            "wv": cm(w_qkv[:, 2 * E + lo:2 * E + lo + P]),
            "bqk": np.ascontiguousarray(
                np.stack([b_qkv[lo:lo + P],
                          b_qkv[E + lo:E + lo + P]], axis=1)
                .astype(np.float32)),
            "bv": np.ascontiguousarray(
                b_qkv[2 * E + lo:2 * E + lo + P].astype(np.float16)
                .reshape(1, P)),
            "wp": np.ascontiguousarray(w_proj[lo:lo + P, :].astype(np.float16)),
        })
    return in_maps


def run_sharded(inputs, trace=False, **kw):
    nc = _build()
    in_maps = make_in_maps(inputs["x"], inputs["w_qkv"], inputs["b_qkv"],
                           inputs["w_proj"])
    res = run_bass_kernel_spmd(nc, in_maps, list(range(N_CORES)), trace=trace,
                               **kw)
    partial = np.zeros((TOK, E), dtype=np.float32)
    for i in range(N_CORES):
        partial += res.results[i]["out"]
    out = partial + np.asarray(inputs["b_proj"], dtype=np.float32)[None, :]
    return out.reshape(B, T, E), res


def kernel(**inputs) -> np.ndarray:
    out, _ = run_sharded(inputs, trace=False)
    return out



# revision 68
# speedup vs baseline: 1.0691x; 1.0292x over previous
"""Trainium2 Bass kernel: 16-head causal self-attention block (QKV proj ->
causal MHA -> output proj), tensor-parallel over heads across 8 NeuronCores.

Contract: kernel(**inputs) takes FULL unsharded inputs
  x      [2, 2048, 1024] f32
  w_qkv  [1024, 3072] f32, b_qkv [3072] f32
  w_proj [1024, 1024] f32, b_proj [1024] f32
and returns the FULL output [2, 2048, 1024] f32.

Sharding: head-parallel. Core c owns global heads (2c, 2c+1):
  - column-parallel QKV (each core takes its 128 q/k/v feature columns)
  - full causal attention for its 2 heads (both batches)
  - row-parallel output projection -> partial [4096, 1024] sums
  - host reduces the 8 partials and adds b_proj.

v2 dataflow (vs v1): software-pipelined emission interleaves QKV of
super-tile s with attention of super s-1 so the PE never starves while
ScalarE runs softmax exp (ScalarE does ONLY exp + a few psum drains; no
Ln -> no act-table thrash). q/k/P/v all fp16 on the PE. v is computed in
natural [token, feat] orientation directly (xT stationary), killing the
PE transposes; v bias folds in at the PSUM drain via a precomputed
broadcast tile. x is pre-transposed on the host so all x loads are plain
contiguous DMAs.

v3 (vs v2, 217.7us -> ~168.5us):
- 1/Z via single-op DVE reciprocal_approx_fast on an SBUF bounce of the
  replicated Z ones-rows of the AV PSUM accumulator (drops the
  Z-broadcast matmul and 30us of exact DVE reciprocal from the
  normalize critical path; the custom op cannot read PSUM directly).
- q/k bias drains moved to ScalarE activation(Identity, bias=AP): DVE
  decongests and the next attention tile's S can start sooner.
- per-tt v PSUM tiles double-buffer the v chains; causal mask-muls on
  the Pool engine (SBUF-only op); all proj drains on DVE.
- output partials drain as fp16 (halves the out-DMA that otherwise
  piles up at the tail).
- tail: supers 4,5 proj deferred into the exp-paced tail via poolY
  chunk drains; super 6's proj is a reserve pulled inline by the last
  q-tile, which pipelines its normalize+proj in 256-col halves with
  Z copies on the idle ScalarE and DMA spread over two queues. This
  keeps the PE dense enough that the HAM clock holds k=8 until the
  final matmul retires.
"""

import numpy as np
from contextlib import ExitStack

import concourse.bass as bass
import concourse.tile as tile
from concourse import bacc, mybir
from concourse.bass_utils import run_bass_kernel_spmd
from concourse.masks import make_upper_triangular

F32 = mybir.dt.float32
F32R = mybir.dt.float32r
F16 = mybir.dt.float16
AF = mybir.ActivationFunctionType

N_CORES = 8
B, T, E, H, D = 2, 2048, 1024, 16, 64
TOK = B * T          # 4096 tokens
P = 128              # partitions
SUPER = 512          # tokens per QKV super-tile
NS = TOK // SUPER    # 8 super-tiles
KCH = E // P         # 8 contraction chunks
QTL = 512            # attention q-tile width
NQT = T // QTL       # 4 q-tiles per batch
KBL = 128            # attention k-block height
VAW = 256            # v_aug cols per token tile: 2 heads x (64 v + 64 ones);
                     # the 64 replicated ones-columns make the AV matmul emit
                     # the softmax denominator Z broadcast on partitions 64-127


def r(ap):
    return ap.bitcast(F32R)


def _emit(nc, tc, ctx):
    # weights arrive host-relayouted: [P, E] chunk-major, one contiguous DMA
    xT_h = nc.declare_dram_parameter("xT", [E, TOK], F16, isOutput=False)
    wq_h = nc.declare_dram_parameter("wq", [P, E], F16, isOutput=False)
    wk_h = nc.declare_dram_parameter("wk", [P, E], F16, isOutput=False)
    wv_h = nc.declare_dram_parameter("wv", [P, E], F16, isOutput=False)
    bqk_h = nc.declare_dram_parameter("bqk", [P, 2], F32, isOutput=False)
    bv_h = nc.declare_dram_parameter("bv", [1, P], F16, isOutput=False)
    wp_h = nc.declare_dram_parameter("wp", [P, E], F16, isOutput=False)
    out_h = nc.declare_dram_parameter("out", [TOK, E], F16, isOutput=True)

    outr = out_h[:].rearrange("(n p) e -> n p e", p=P)  # [32, 128, 1024]

    # ---------------- persistent tiles ----------------
    const = ctx.enter_context(tc.tile_pool(name="const", bufs=1))
    mask_tri = const.tile([P, P], F16)  # mask[p, f] = 1.0 iff p <= f
    make_upper_triangular(nc, mask_tri[:], val=1.0, diag=True)
    ones1h = const.tile([1, P], F16)
    nc.vector.memset(ones1h[:], 1.0)


    # weight/bias loads go on the Activation queue (idle at startup) so the
    # x loads on the sync queue aren't delayed behind them
    bqk_sb = const.tile([P, 2], F32)
    bvr = const.tile([1, P], F16)
    nc.scalar.dma_start(bqk_sb[:], bqk_h[:])
    nc.scalar.dma_start(bvr[:], bv_h[:])
    bq_sb = bqk_sb[:, 0:1]
    bk_sb = bqk_sb[:, 1:2]
    wq_sb = const.tile([P, E], F16)
    wk_sb = const.tile([P, E], F16)
    wv_sb = const.tile([P, E], F16)
    wp_sb = const.tile([P, E], F16)
    # one weight per queue: a single queue moves these serially at ~100GB/s
    # and the first q-chain would wait ~10us for wq otherwise
    for wsb, wh in ((wq_sb, wq_h), (wk_sb, wk_h), (wv_sb, wv_h),
                    (wp_sb, wp_h)):
        nc.scalar.dma_start(wsb[:], wh[:])
    bvb = const.tile([P, P], F32)  # v bias broadcast to all 128 partitions

    persist = ctx.enter_context(tc.tile_pool(name="persist", bufs=1))

    with ExitStack() as ph:
        xpool = ph.enter_context(tc.tile_pool(name="xp", bufs=4))
        ptpool = ph.enter_context(tc.tile_pool(name="pTp", bufs=12))
        zbpool = ph.enter_context(tc.tile_pool(name="zbp", bufs=2))
        opool = ph.enter_context(tc.tile_pool(name="op", bufs=6))
        poolQ = ph.enter_context(tc.tile_pool(name="poolQ", bufs=2, space="PSUM"))
        poolS = ph.enter_context(tc.tile_pool(name="poolS", bufs=2, space="PSUM"))
        poolY = ph.enter_context(tc.tile_pool(name="poolY", bufs=2, space="PSUM"))

        # v bias broadcast: bvb[p, j] = bv[j] via rank-1 matmul of ones x bv
        pb = poolQ.tile([P, P], F32, tag="q", name="pb")
        # HAM warmup: the PE would otherwise idle ~6us waiting for the weight
        # DMAs and start the real QKV chains at half clock (K=4/8). A burst
        # of dependency-free matmuls (ones x ones, overwritten below) keeps
        # the PE activity window busy so the clock-gate opens first.
        for _ in range(48):
            nc.tensor.matmul(pb[0:32, :], lhsT=ones1h[0:1, 0:32],
                             rhs=ones1h[:], start=True, stop=True)
        nc.tensor.matmul(pb[:], lhsT=ones1h[:], rhs=bvr[:], start=True,
                         stop=True)
        nc.vector.tensor_copy(bvb[:], pb[:])

        yts = [persist.tile([P, QTL], F16, tag=f"yt{n}", name="yt")
               for n in range(NS)]
        projq = []
        qTs, kTs, vas = [], [], []
        for s in range(NS):
            qTs.append(persist.tile([P, SUPER], F16, tag=f"qT{s}", name="qTt"))
            kTs.append(persist.tile([P, SUPER], F16, tag=f"kT{s}", name="kTt"))
            vat = persist.tile([P, 4 * VAW], F16, tag=f"va{s}", name="vat")
            vas.append(vat)
            # whole tile starts as ones; the per-super v drains overwrite the
            # v columns, leaving the replicated ones-columns (64-127 of each
            # head group) that make the AV matmul emit Z on partitions 64-127
            nc.gpsimd.memset(vat[:], 1.0)

        xtiles = {}
        xT_cm = xT_h[:].rearrange("(c p) tok -> p c tok", p=P)  # [128,8,4096]

        def issue_x(s):
            # one strided DMA per super-tile; chunk ch lands contiguous at
            # cols [ch*512, (ch+1)*512). The first super is split in half so
            # its q-chain can start on chunks 0-3 while 4-7 are in flight.
            xt = xpool.tile([P, KCH * SUPER], F16, tag="xT", name="xTt")
            nc.sync.dma_start(
                xt[:].rearrange("p (c t) -> p c t", c=KCH),
                xT_cm[:, :, s * SUPER:(s + 1) * SUPER])
            xtiles[s] = [xt[:, ch * SUPER:(ch + 1) * SUPER]
                         for ch in range(KCH)]

        issue_x(0)
        issue_x(1)
        issue_x(2)

        def qkv_units(s):
            xt = xtiles[s]
            if s + 3 < NS:
                issue_x(s + 3)
            yield
            # q chain fully before k chain: pfq stops ~2us earlier, so its
            # ScalarE drain overlaps the k chain and the poolQ slot is
            # already free when the v chains (and the next super's q) need
            # it -- this was the recurring super-boundary PE stall
            pfq = poolQ.tile([P, SUPER], F32, tag="q", name="pfq")
            for ch in range(KCH):
                nc.tensor.matmul(
                    pfq[:], lhsT=wq_sb[:, ch * P:(ch + 1) * P],
                    rhs=xt[ch][:], start=(ch == 0), stop=(ch == KCH - 1))
                if ch % 2 == 1:
                    yield
            # drain on ScalarE (bias folds into the activation) so the
            # attention S-start never queues behind DVE
            nc.scalar.activation(qTs[s][:], pfq[:], AF.Identity, bias=bq_sb)
            pfk = poolQ.tile([P, SUPER], F32, tag="q", name="pfk")
            for ch in range(KCH):
                nc.tensor.matmul(
                    pfk[:], lhsT=wk_sb[:, ch * P:(ch + 1) * P],
                    rhs=xt[ch][:], start=(ch == 0), stop=(ch == KCH - 1))
                if ch % 2 == 1:
                    yield
            nc.scalar.activation(kTs[s][:], pfk[:], AF.Identity, bias=bk_sb)
            yield
            bvb2 = bvb[:].rearrange("p (h d) -> p h d", h=2)
            for tt in range(4):
                vps = poolQ.tile([P, P], F32, tag="q", name="vps")
                for ch in range(KCH):
                    nc.tensor.matmul(
                        vps[:],
                        lhsT=xt[ch][:, tt * P:(tt + 1) * P],
                        rhs=wv_sb[:, ch * P:(ch + 1) * P],
                        start=(ch == 0), stop=(ch == KCH - 1))
                dst = vas[s][:, tt * VAW:(tt + 1) * VAW].rearrange(
                    "p (h x) -> p h x", x=2 * D)[:, :, 0:D]
                src = vps[:].rearrange("p (h d) -> p h d", h=2)
                nc.vector.tensor_add(dst, src, bvb2)
                yield

        projq = []

        def attn_units(b, qi):
            nkb = 4 * qi + 4   # k blocks of 128 covering [0, (qi+1)*512)
            sq = 4 * b + qi    # super-tile holding this q range
            # the last q-tile takes its accumulators from poolQ (idle once
            # QKV is done) so its blocks overlap the previous q-tile's tail
            ypool, ytag = (poolQ, "q") if sq == NS - 1 else (poolY, "y")
            pys = [ypool.tile([P, QTL], F32, tag=ytag, name=f"py{h}")
                   for h in range(2)]

            def emit_S(kb):
                c0 = max(0, kb * KBL - qi * QTL)
                sk, kc = 4 * b + kb // 4, (kb % 4) * KBL
                ps = poolS.tile([P, 2 * QTL], F32, tag="s", name="ps")
                for h in range(2):
                    nc.tensor.matmul(
                        ps[:, h * QTL + c0:(h + 1) * QTL],
                        lhsT=kTs[sk][64 * h:64 * h + 64, kc:kc + KBL],
                        rhs=qTs[sq][64 * h:64 * h + 64, c0:QTL],
                        start=True, stop=True)
                return ps, c0

            cur = emit_S(0)
            for kb in range(nkb):
                ps, c0 = cur
                if kb + 1 < nkb:
                    cur = emit_S(kb + 1)
                pt = ptpool.tile([P, 2 * QTL], F16, tag="pT", name="pt")
                if c0 == 0:
                    nc.scalar.activation(pt[:], ps[:], AF.Exp, scale=0.125)
                else:
                    src = ps[:].rearrange("p (h q) -> p h q", h=2)[:, :, c0:]
                    dst = pt[:].rearrange("p (h q) -> p h q", h=2)[:, :, c0:]
                    nc.scalar.activation(dst, src, AF.Exp, scale=0.125)
                if kb * KBL >= qi * QTL:  # diagonal block: causal triangle
                    sl = pt[:].rearrange("p (h q) -> p h q",
                                         h=2)[:, :, c0:c0 + P]
                    m3 = mask_tri[:].rearrange(
                        "p (u f) -> p u f", u=1).broadcast_to([P, 2, P])
                    # DVE runs this 2x faster than Pool (351 vs 670ns) and
                    # it sits on the exp->mask->AV chain of diagonal blocks;
                    # front-loading keeps the DVE queue clear here
                    nc.vector.tensor_mul(sl, sl, m3)
                vo = (kb % 4) * VAW
                sk = 4 * b + kb // 4
                for h in range(2):
                    nc.tensor.matmul(
                        pys[h][:, c0:QTL],
                        lhsT=vas[sk][:, vo + 2 * D * h:vo + 2 * D * h + 2 * D],
                        rhs=pt[:, h * QTL + c0:(h + 1) * QTL],
                        start=(kb == 0), stop=(kb == nkb - 1))
                yield
            # normalize: y * (1/Z). pys[h] partitions 64-127 already hold Z
            # broadcast (the replicated ones-columns), so a fast approximate
            # reciprocal reads them straight out of PSUM -- no Z-broadcast
            # matmul, no exact (slow) reciprocal on the critical path. The
            # custom-DVE recip needs raw fp32 bits and PSUM reads break its
            # BITWISE_NOT seed, so bounce Z through SBUF first.
            yt = yts[sq]
            if sq == NS - 1:
                # last q-tile: nothing overlaps its tail, so pipeline
                # normalize+proj in 256-col halves to keep the PE fed and
                # the final out-DMA issued as early as possible. PE filler
                # during the Z-drain latency comes from the reserve (the
                # last deferred super's proj units); the Z copies go on
                # ScalarE, which is idle once the final exp retires.
                def pull(n):
                    while n > 0 and reserve:
                        try:
                            next(reserve[0])
                            n -= 1
                        except StopIteration:
                            reserve.pop(0)
                pull(3)
                for half in range(2):
                    hs = slice(half * 2 * P, (half + 1) * 2 * P)
                    zs = zbpool.tile([P, 2 * P], F32, tag="zb", name="zs")
                    # one Z copy per engine so they run in parallel
                    nc.vector.tensor_copy(zs[0:D, :], pys[0][D:2 * D, hs])
                    nc.scalar.activation(zs[D:2 * D, :], pys[1][D:2 * D, hs],
                                         AF.Copy)
                    zinv = zbpool.tile([P, 2 * P], F32, tag="zb", name="zinv")
                    nc.vector.reciprocal_approx_fast(zinv[:], zs[:])
                    nc.vector.tensor_mul(yt[0:D, hs], pys[0][0:D, hs],
                                         zinv[0:D, :])
                    nc.vector.tensor_mul(yt[D:2 * D, hs], pys[1][0:D, hs],
                                         zinv[D:2 * D, :])
                    pull(1)
                    yield
                    for tt4 in (2 * half, 2 * half + 1):
                        cs = slice(tt4 * P, (tt4 + 1) * P)
                        pos = poolS.tile([P, E], F32, tag="s", name="po")
                        for oc in range(2):
                            nc.tensor.matmul(
                                pos[:, oc * 512:(oc + 1) * 512],
                                lhsT=yt[:, cs],
                                rhs=wp_sb[:, oc * 512:(oc + 1) * 512],
                                start=True, stop=True)
                        ti = (b * T + qi * QTL) // P + tt4
                        ot = opool.tile([P, E], F16, tag="ot", name="ot")
                        # drain halves on DVE + ScalarE (idle after the
                        # final exp) in parallel, and DMA each half as soon
                        # as its drain lands, on separate queues -- the
                        # kernel cannot retire until the last byte is out
                        nc.vector.tensor_copy(ot[:, 0:512], pos[:, 0:512])
                        nc.scalar.activation(ot[:, 512:1024],
                                             pos[:, 512:1024], AF.Copy)
                        nc.gpsimd.dma_start(outr[ti][:, 0:512],
                                            ot[:, 0:512])
                        nc.sync.dma_start(outr[ti][:, 512:1024],
                                          ot[:, 512:1024])
                        yield
                return
            zs = zbpool.tile([P, QTL], F32, tag="zb", name="zs")
            nc.vector.tensor_copy(zs[0:D, :], pys[0][D:2 * D, :])
            nc.vector.tensor_copy(zs[D:2 * D, :], pys[1][D:2 * D, :])
            zinv = zbpool.tile([P, QTL], F32, tag="zb", name="zinv")
            nc.vector.reciprocal_approx_fast(zinv[:], zs[:])
            yield
            nc.vector.tensor_mul(yt[0:D, :], pys[0][0:D, :], zinv[0:D, :])
            nc.vector.tensor_mul(yt[D:2 * D, :], pys[1][0:D, :],
                                 zinv[D:2 * D, :])
            yield
            if sq in (4, 5):
                # deferred: dense PE work reserved for the exp-paced tail so
                # the HAM clock stays warm through the last q-tiles
                projq.append(proj_units(b, qi))
            elif sq == 6:
                # super 6's proj is the reserve the last q-tile pulls from
                reserve.append(proj_units(b, qi, tail=True))
            else:
                yield from proj_units(b, qi)

        def proj_units(b, qi, tail=False):
            yt = yts[4 * b + qi]
            # mid-tail-deferred tiles take their PSUM from poolY (idle in
            # the tail: the last q-tile's accumulators live in poolQ), so
            # the tail's S-block double-buffer in poolS never blocks on a
            # proj drain. The reserve tiles pulled during the last q-tile's
            # normalize use poolS (S-blocks are done by then) and drain in
            # halves on DVE+ScalarE so they never queue in front of the
            # normalize chain on DVE.
            deferred = 4 * b + qi in (4, 5)
            for tt4 in range(4):
                cs = slice(tt4 * P, (tt4 + 1) * P)
                ti = (b * T + qi * QTL) // P + tt4
                ot = opool.tile([P, E], F16, tag="ot", name="ot")
                if tail:
                    pos = poolS.tile([P, E], F32, tag="s", name="po")
                    for oc in range(2):
                        nc.tensor.matmul(
                            pos[:, oc * 512:(oc + 1) * 512],
                            lhsT=yt[:, cs],
                            rhs=wp_sb[:, oc * 512:(oc + 1) * 512],
                            start=True, stop=True)
                    nc.vector.tensor_copy(ot[:, 0:512], pos[:, 0:512])
                    nc.scalar.activation(ot[:, 512:1024], pos[:, 512:1024],
                                         AF.Copy)
                    dq = nc.sync if tt4 % 2 else nc.gpsimd
                    dq.dma_start(outr[ti], ot[:])
                elif deferred:
                    for oc in range(2):
                        poc = poolY.tile([P, QTL], F32, tag="y", name="poc")
                        nc.tensor.matmul(
                            poc[:], lhsT=yt[:, cs],
                            rhs=wp_sb[:, oc * 512:(oc + 1) * 512],
                            start=True, stop=True)
                        nc.vector.tensor_copy(
                            ot[:, oc * 512:(oc + 1) * 512], poc[:])
                    nc.gpsimd.dma_start(outr[ti], ot[:])
                else:
                    pos = poolS.tile([P, E], F32, tag="s", name="po")
                    for oc in range(2):
                        nc.tensor.matmul(
                            pos[:, oc * 512:(oc + 1) * 512],
                            lhsT=yt[:, cs],
                            rhs=wp_sb[:, oc * 512:(oc + 1) * 512],
                            start=True, stop=True)
                    nc.vector.tensor_copy(ot[:], pos[:])
                    nc.gpsimd.dma_start(outr[ti], ot[:])
                yield

        # ---- software-pipelined emission driver ----
        from collections import deque
        pending = deque()
        backlog = [0]
        rr = [0]
        reserve = []

        def pump(n):
            # round-robin across active attention generators so a finishing
            # q-tile's normalize tail interleaves with the next q-tile's
            # S blocks in every engine FIFO
            while n > 0 and pending:
                idx = rr[0] % len(pending)
                g = pending[idx]
                try:
                    next(g)
                    backlog[0] -= 1
                    n -= 1
                    rr[0] = idx + 1
                except StopIteration:
                    pending.remove(g)

        QU = 14  # units per qkv super (1 issue + 8 chunks + 1 drain + 4 v)
        for s in range(NS):
            # pace the attention backlog evenly across this super's qkv units
            # so the ScalarE exp stream never starves at a super boundary
            # keep a couple of attention units in reserve so the PE engine
            # queue never runs dry at the super boundary (the reserve drains
            # while the new super's q/k chains wait on their x DMA / weights).
            # Front-load the pumping into the q/k chunk phase (units 1..10):
            # the attention tail's DVE chain (zs copies, recip, muls) then
            # executes while the PE runs the independent q/k chains, instead
            # of queueing in front of the v-adds that gate the next super's
            # PSUM slots.
            start_backlog = max(0, backlog[0] - 2)
            done, k = 0, 0
            for u in qkv_units(s):
                k += 1
                want = min(start_backlog, (start_backlog * k + 11) // 12)
                while done < want and pending:
                    pump(1)
                    done += 1
            b, qi = divmod(s, NQT)
            pending.append(attn_units(b, qi))
            backlog[0] += (4 * qi + 4) + 6
        while pending or projq:
            while projq:
                pending.append(projq.pop(0))
            pump(1)
        while reserve:
            try:
                next(reserve[0])
            except StopIteration:
                reserve.pop(0)


_NC_CACHE = None


def _build():
    global _NC_CACHE
    if _NC_CACHE is None:
        nc = bacc.Bacc("TRN2", target_bir_lowering=False, debug=False)
        with tile.TileContext(nc) as tc:
            with ExitStack() as ctx:
                _emit(nc, tc, ctx)
        nc.compile()
        _NC_CACHE = nc
    return _NC_CACHE


def make_in_maps(x, w_qkv, b_qkv, w_proj):
    x2 = np.asarray(x, dtype=np.float32).reshape(TOK, E).astype(np.float16)
    xT = np.ascontiguousarray(x2.T)  # [E, TOK] feature-major
    w_qkv = np.asarray(w_qkv, dtype=np.float32)
    b_qkv = np.asarray(b_qkv, dtype=np.float32)
    w_proj = np.asarray(w_proj, dtype=np.float32)
    def cm(w):  # [E, P] slice -> [P, E] chunk-major fp16
        return np.ascontiguousarray(
            w.astype(np.float16).reshape(KCH, P, P).transpose(1, 0, 2)
            .reshape(P, E))

    in_maps = []
    for c in range(N_CORES):
        lo = P * c
        in_maps.append({
            "xT": xT,
            "wq": cm(w_qkv[:, lo:lo + P]),
            "wk": cm(w_qkv[:, E + lo:E + lo + P]),
            "wv": cm(w_qkv[:, 2 * E + lo:2 * E + lo + P]),
            "bqk": np.ascontiguousarray(
                np.stack([b_qkv[lo:lo + P],
                          b_qkv[E + lo:E + lo + P]], axis=1)
                .astype(np.float32)),
            "bv": np.ascontiguousarray(
                b_qkv[2 * E + lo:2 * E + lo + P].astype(np.float16)
                .reshape(1, P)),
            "wp": np.ascontiguousarray(w_proj[lo:lo + P, :].astype(np.float16)),
        })
    return in_maps


def run_sharded(inputs, trace=False, **kw):
    nc = _build()
    in_maps = make_in_maps(inputs["x"], inputs["w_qkv"], inputs["b_qkv"],
                           inputs["w_proj"])
    res = run_bass_kernel_spmd(nc, in_maps, list(range(N_CORES)), trace=trace,
                               **kw)
    partial = np.zeros((TOK, E), dtype=np.float32)
    for i in range(N_CORES):
        partial += res.results[i]["out"]
    out = partial + np.asarray(inputs["b_proj"], dtype=np.float32)[None, :]
    return out.reshape(B, T, E), res


def kernel(**inputs) -> np.ndarray:
    out, _ = run_sharded(inputs, trace=False)
    return out

